# revision 1
# baseline (speedup 1.0000x reference)
"""Causal multi-head attention block (LN + rotary QKV + causal attention +
out-projection) on 8 Trainium2 NeuronCores.

Sharding: data-parallel over batch (b=2), tensor-parallel over heads
(16 heads -> 4 per core). Core c handles batch c//4, heads 4*(c%4)..+4.
Each core computes a partial out-projection (row-parallel w_out); the host
sums the 4 partials per batch.

Per-core pipeline:
  - x arrives both natural (for LN stats via bn_stats) and host-pretransposed
    (xT, the QKV stationary operand), so no on-device transpose of x.
  - LN is folded around the QKV matmul: qkv = rstd*(x@W - mu*colsum(W)),
    with rstd further folded into the rotary cos/sin tiles and V copy.
  - rotary on DVE: q/k features are pair-interleaved so rotate_half is an
    adjacent-pair swap (negative-step AP view).
  - attention: S^T = K_j Q_c^T blocks (fp32r, k on partitions), exp on ACT
    with no max subtraction (logits bounded), causal masking via one gpsimd
    affine_select on the bf16 P tile, PV in bf16 with a ones column on V so
    the softmax denominator comes out of the same matmul, normalization via
    ln/exp reciprocal on ACT (single activation table set).
"""
import sys
import os
import numpy as np
from contextlib import ExitStack

sys.path.insert(0, '/opt/trn_rl_repo')
if '/root/.axon_site' not in sys.path:
    sys.path.insert(0, '/root/.axon_site')

import concourse.bass as bass
import concourse.tile as tile
from concourse import mybir, bacc
from concourse.bass_utils import run_bass_kernel_spmd
from concourse.masks import make_identity

F32 = mybir.dt.float32
F32R = mybir.dt.float32r
BF16 = mybir.dt.bfloat16
EXPF = mybir.ActivationFunctionType.Exp
LNF = mybir.ActivationFunctionType.Ln

N = 2048          # sequence length
D = 1024          # model dim
DH = 64           # head dim
NT = N // 128     # 16 token tiles
NCH = N // 512    # 4 q-chunks
LN_EPS = 1e-5

_cache = {}


def _patch_act_tables():
    """Keep Exp and Ln only in natural_log_exp_and_others so the table-load
    inserter can't ping-pong between exp_and_others and natural_log."""
    if _cache.get('act_patched'):
        return
    import concourse.bacc as bacc_mod
    orig = bacc_mod.get_activation_tables

    def patched(arch):
        t = dict(orig(arch))
        out = {}
        for name, fns in t.items():
            fns = set(fns)
            if name != 'natural_log_exp_and_others':
                fns.discard(mybir.ActivationFunctionType.Exp)
                fns.discard(mybir.ActivationFunctionType.Ln)
            out[name] = fns
        return out

    bacc_mod.get_activation_tables = patched
    _cache['act_patched'] = True


def _ap(t, off, dims):
    """Free-dim view of tile t at free-offset off with custom free dims."""
    return bass.AP(tensor=t.tensor, offset=t.offset + off, ap=[t.ap[0]] + dims)


def build():
    _patch_act_tables()
    nc = bacc.Bacc()
    x_d = nc.declare_dram_parameter("x", [N, D], F32, isOutput=False)
    xT_d = nc.declare_dram_parameter("xT", [D, N], F32R, isOutput=False)
    wqk_d = nc.declare_dram_parameter("wqk", [D, 512], F32R, isOutput=False)
    wv_d = nc.declare_dram_parameter("wv", [D, 256], F32R, isOutput=False)
    wo_d = nc.declare_dram_parameter("wo", [256, D], F32R, isOutput=False)
    trig_d = nc.declare_dram_parameter("trig", [N, 2 * DH], F32, isOutput=False)
    cqkv_d = nc.declare_dram_parameter("cqkv", [1, 768], F32, isOutput=False)
    y_d = nc.declare_dram_parameter("y", [N, D], F32, isOutput=True)

    xT_r = xT_d.rearrange("(k p) t -> p k t", p=128)

    with tile.TileContext(nc) as tc:
        with ExitStack() as cx:
            const = cx.enter_context(tc.tile_pool(name="const", bufs=1))
            big = cx.enter_context(tc.tile_pool(name="big", bufs=1))

            wqk = const.tile([128, 8, 512], F32R)
            nc.sync.dma_start(out=wqk[:], in_=wqk_d.rearrange("(k p) f -> p k f", p=128))
            wv = const.tile([128, 8, 256], F32R)
            nc.sync.dma_start(out=wv[:], in_=wv_d.rearrange("(k p) f -> p k f", p=128))
            wo = const.tile([128, 2, 1024], F32R)
            nc.sync.dma_start(out=wo[:], in_=wo_d.rearrange("(g p) f -> p g f", p=128))
            trig = const.tile([128, NT, 2 * DH], F32)
            nc.sync.dma_start(out=trig[:], in_=trig_d.rearrange("(j p) d -> p j d", p=128))
            ident = const.tile([128, 128], F32)
            make_identity(nc, ident[:])
            eps_t = const.tile([128, 1], F32)
            nc.vector.memset(eps_t[:], LN_EPS)
            cqkv_row = const.tile([1, 768], F32)
            nc.sync.dma_start(out=cqkv_row[:], in_=cqkv_d[:])
            cqkv_b = const.tile([128, 768], F32)
            nc.gpsimd.partition_broadcast(cqkv_b[:], cqkv_row[:])
            # tri[k, i] = 0 if i >= k else -1000  (causal bias for the
            # diagonal 128-strip, same for every diagonal block)
            tri = const.tile([128, 128], F32)
            nc.gpsimd.memset(tri[:], 0.0)
            nc.gpsimd.affine_select(out=tri[:], in_=tri[:],
                                    compare_op=mybir.AluOpType.is_ge,
                                    fill=-1000.0, base=0,
                                    pattern=[[1, 128]], channel_multiplier=-1)

            # persistent activations
            qT = big.tile([128, 2, N], F32R)     # [2 heads x 64 d, pair, tok]
            kT = big.tile([128, 2, N], F32R)
            vA = big.tile([128, NT, 4, DH + 1], BF16)   # V_ext, ones col 64
            oT = big.tile([128, 2, N], F32R)     # attention out^T per pair
            nc.gpsimd.memset(vA[:, :, :, DH:DH + 1], 1.0)

            # ---------------- Phase A: LN + QKV + rotary -----------------
            with ExitStack() as ca:
                pa = ca.enter_context(tc.tile_pool(name="pa", bufs=3))
                st = ca.enter_context(tc.tile_pool(name="st", bufs=4))
                tps = ca.enter_context(tc.tile_pool(name="tps", bufs=3, space="PSUM"))
                qkps = ca.enter_context(tc.tile_pool(name="qkps", bufs=2, space="PSUM"))
                vps = ca.enter_context(tc.tile_pool(name="vps", bufs=2, space="PSUM"))

                for j in range(NT):
                    sl = slice(128 * j, 128 * (j + 1))
                    x_t = pa.tile([128, D], F32, tag="x")
                    nc.sync.dma_start(out=x_t[:], in_=x_d[sl, :])
                    xT_t = pa.tile([128, 8, 128], F32R, tag="xT")
                    nc.sync.dma_start(out=xT_t[:], in_=xT_r[:, :, sl])
                    stats = st.tile([128, 2, 6], F32, tag="stats")
                    nc.vector.bn_stats(out=stats[:, 0, :], in_=x_t[:, 0:512])
                    nc.vector.bn_stats(out=stats[:, 1, :], in_=x_t[:, 512:1024])
                    mv = st.tile([128, 2], F32, tag="mv")
                    nc.vector.bn_aggr(out=mv[:], in_=stats[:])
                    # rstd = exp(-0.5*ln(var+eps))
                    lnv = st.tile([128, 1], F32, tag="lnv")
                    nc.scalar.activation(out=lnv[:], in_=mv[:, 1:2], func=LNF, bias=eps_t[:])
                    rstd = st.tile([128, 1], F32, tag="rstd")
                    nc.scalar.activation(out=rstd[:], in_=lnv[:], func=EXPF, scale=-0.5)
                    # QKV matmuls on raw xT (fp32r)
                    qk_ps = qkps.tile([128, 512], F32, tag="qkp", name="qkp")
                    for k in range(8):
                        nc.tensor.matmul(qk_ps[:], xT_t[:, k, :], wqk[:, k, :],
                                         start=(k == 0), stop=(k == 7))
                    v_ps = vps.tile([128, 256], F32, tag="vp", name="vp")
                    for k in range(8):
                        nc.tensor.matmul(v_ps[:], xT_t[:, k, :], wv[:, k, :],
                                         start=(k == 0), stop=(k == 7))
                    # LN correction: qkv = rstd*(raw - mu*colsum); rstd folded
                    # into cos/sin (q,k) and the V copy.
                    t2 = st.tile([128, 768], F32, tag="t2", name="t2")
                    nc.vector.tensor_scalar(out=t2[:], in0=cqkv_b[:],
                                            scalar1=mv[:, 0:1], scalar2=None,
                                            op0=mybir.AluOpType.mult)
                    qkv_c = pa.tile([128, 768], F32, tag="qkvc", name="qkvc")
                    nc.vector.tensor_tensor(out=qkv_c[:, 0:512], in0=qk_ps[:],
                                            in1=t2[:, 0:512],
                                            op=mybir.AluOpType.subtract)
                    nc.vector.tensor_tensor(out=qkv_c[:, 512:768], in0=v_ps[:],
                                            in1=t2[:, 512:768],
                                            op=mybir.AluOpType.subtract)
                    qk_c = qkv_c[:, 0:512]
                    nc.vector.tensor_scalar(out=vA[:, j, :, 0:DH],
                                            in0=qkv_c[:, 512:768].rearrange("p (h d) -> p h d", d=DH),
                                            scalar1=rstd[:], scalar2=None,
                                            op0=mybir.AluOpType.mult)
                    # rstd-scaled rotary coefficient tiles (cos|sin packed)
                    cs_ss = st.tile([128, 2 * DH], F32, tag="css", name="css")
                    nc.vector.tensor_scalar(out=cs_ss[:], in0=trig[:, j, :],
                                            scalar1=rstd[:], scalar2=None,
                                            op0=mybir.AluOpType.mult)
                    # rotary: qk_rot = qk_c*cos + swap_adj(qk_c)*sin
                    cos_b = _ap(cs_ss, 0, [[0, 8], [1, DH]])
                    sin_b = _ap(cs_ss, DH, [[0, 8], [2, 32], [1, 2]])
                    t_cos = pa.tile([128, 512], F32, tag="tcos", name="tcos")
                    nc.vector.tensor_tensor(
                        out=t_cos[:].rearrange("p (g d) -> p g d", d=DH),
                        in0=qk_c.rearrange("p (g d) -> p g d", d=DH),
                        in1=cos_b, op=mybir.AluOpType.mult)
                    t_sin = pa.tile([128, 512], F32, tag="tsin", name="tsin")
                    qk_swap = _ap(qkv_c, 1, [[DH, 8], [2, 32], [-1, 2]])
                    nc.vector.tensor_tensor(
                        out=t_sin[:].rearrange("p (g i t) -> p g i t", g=8, t=2),
                        in0=qk_swap, in1=sin_b, op=mybir.AluOpType.mult)
                    qk_rot = pa.tile([128, 512], F32, tag="qkr", name="qkr")
                    nc.gpsimd.tensor_tensor(out=qk_rot[:], in0=t_cos[:], in1=t_sin[:],
                                            op=mybir.AluOpType.add)
                    # transpose q,k chunks -> qT/kT (f: 0=q pair0, 1=q pair1,
                    # 2=k pair0, 3=k pair1)
                    for f in range(4):
                        tp = tps.tile([128, 128], F32, tag="tp")
                        nc.tensor.transpose(tp[:], qk_rot[:, 128 * f:128 * (f + 1)], ident[:])
                        dst = qT if f < 2 else kT
                        pair = f % 2
                        if f % 2 == 0:
                            nc.vector.tensor_copy(out=dst[:, pair, sl], in_=tp[:])
                        else:
                            nc.scalar.copy(out=dst[:, pair, sl], in_=tp[:])

            # ---------------- Phase B: attention ------------------------
            # Both heads of a pair share one 2-bank S psum tile and one exp;
            # PV lags S by one j-step so the PE doesn't queue behind exp.
            with ExitStack() as cb:
                pb = cb.enter_context(tc.tile_pool(name="pb", bufs=6))
                nrm = cb.enter_context(tc.tile_pool(name="nrm", bufs=2))
                pc_ = cb.enter_context(tc.tile_pool(name="pc", bufs=4))
                sps = cb.enter_context(tc.tile_pool(name="sps", bufs=2, space="PSUM"))
                yps = cb.enter_context(tc.tile_pool(name="yps", bufs=2, space="PSUM"))
                ops_ = cb.enter_context(tc.tile_pool(name="ops", bufs=1, space="PSUM"))

                for c in range(NCH):           # q chunk of 512
                    for hp in range(2):        # head pair
                        ot_ps = ops_.tile([DH + 1, 1024], F32, tag="ot", name="ot")
                        njb = 4 * c + 4        # k blocks for this chunk
                        pend = []

                        def emit_pv(pj, pt):
                            for hh in range(2):
                                nc.tensor.matmul(ot_ps[:, 512 * hh:512 * (hh + 1)],
                                                 vA[:, pj, 2 * hp + hh, :],
                                                 pt[:, hh, :],
                                                 start=(pj == 0), stop=(pj == njb - 1),
                                                 skip_group_check=True)

                        for jj in range(njb):
                            dj = jj - 4 * c
                            s_ps = sps.tile([128, 1024], F32, tag="s", name="s")
                            for hh in range(2):
                                bp = 64 * hh
                                nc.tensor.matmul(
                                    s_ps[:, 512 * hh:512 * (hh + 1)],
                                    kT[bp:bp + 64, hp, 128 * jj:128 * (jj + 1)],
                                    qT[bp:bp + 64, hp, 512 * c:512 * (c + 1)],
                                    start=True, stop=True, skip_group_check=True)
                            p_t = pb.tile([128, 2, 512], BF16, tag="p", name="p")
                            if dj < 0:
                                nc.scalar.activation(out=p_t[:], in_=s_ps[:], func=EXPF)
                            else:
                                q0 = 128 * dj
                                nc.scalar.activation(
                                    out=p_t[:, :, q0:512],
                                    in_=s_ps[:].rearrange("p (h q) -> p h q", h=2)[:, :, q0:512],
                                    func=EXPF)
                                nc.gpsimd.affine_select(
                                    out=p_t[:], in_=p_t[:],
                                    compare_op=mybir.AluOpType.is_ge,
                                    fill=0.0, base=-q0,
                                    pattern=[[0, 2], [1, 512]], channel_multiplier=-1)
                            pend.append((jj, p_t))
                            if len(pend) > 2:
                                emit_pv(*pend.pop(0))
                        while pend:
                            emit_pv(*pend.pop(0))
                        # normalize: oT = ot_ps[0:64] * (1/l), l = ot_ps[64]
                        lnl = nrm.tile([1, 1024], F32, tag="lnl", name="lnl")
                        nc.scalar.activation(out=lnl[:], in_=ot_ps[DH:DH + 1, :], func=LNF)
                        rec = nrm.tile([1, 1024], F32, tag="rec", name="rec")
                        nc.scalar.activation(out=rec[:], in_=lnl[:], func=EXPF, scale=-1.0)
                        rec_b = nrm.tile([64, 1024], F32, tag="recb", name="recb")
                        nc.gpsimd.partition_broadcast(rec_b[:], rec[:])
                        for hh in range(2):
                            nc.vector.tensor_tensor(
                                out=oT[64 * hh:64 * (hh + 1), hp, 512 * c:512 * (c + 1)],
                                in0=ot_ps[0:DH, 512 * hh:512 * (hh + 1)],
                                in1=rec_b[:, 512 * hh:512 * (hh + 1)],
                                op=mybir.AluOpType.mult)
                    # out-projection for this chunk's four token tiles
                    for j in range(4 * c, 4 * c + 4):
                        for m in range(2):
                            y_ps = yps.tile([128, 512], F32, tag="yp", name="yp")
                            for hp2 in range(2):
                                nc.tensor.matmul(y_ps[:],
                                                 oT[:, hp2, 128 * j:128 * (j + 1)],
                                                 wo[:, hp2, 512 * m:512 * (m + 1)],
                                                 start=(hp2 == 0), stop=(hp2 == 1))
                            y_sb = pc_.tile([128, 512], F32, tag="ysb", name="ysb")
                            if (j + m) % 2 == 0:
                                nc.vector.tensor_copy(out=y_sb[:], in_=y_ps[:])
                            else:
                                nc.scalar.copy(out=y_sb[:], in_=y_ps[:])
                            nc.sync.dma_start(
                                out=y_d[128 * j:128 * (j + 1), 512 * m:512 * (m + 1)],
                                in_=y_sb[:])

    nc.finalize()
    return nc


def _host_shards(x, rotary_pos_emb, ln_w, ln_b, w_qkv, w_out):
    """Build the 8 per-core input maps."""
    SCALE = DH ** -0.5
    # pair-interleaved feature order within each head: (i, i+32) adjacent
    perm = np.empty(DH, dtype=np.int64)
    perm[0::2] = np.arange(32)
    perm[1::2] = np.arange(32) + 32
    cos = np.cos(rotary_pos_emb).astype(np.float32)     # [N, DH]
    sin = np.sin(rotary_pos_emb).astype(np.float32)
    cosn = cos[:, perm]
    sinn = sin[:, perm].copy()
    sinn[:, 0::2] *= -1.0                               # -sin on even slots
    trig = np.ascontiguousarray(np.concatenate([cosn, sinn], axis=1))

    lw = np.asarray(ln_w, dtype=np.float32)[:, None]
    w_q = (np.asarray(w_qkv[:, 0:1024]) * SCALE * lw).astype(np.float32)
    w_k = (np.asarray(w_qkv[:, 1024:2048]) * lw).astype(np.float32)
    w_v = (np.asarray(w_qkv[:, 2048:3072]) * lw).astype(np.float32)
    if np.abs(np.asarray(ln_b)).max() != 0:
        raise NotImplementedError("nonzero ln_b not supported by this kernel")

    in_maps = []
    for core in range(8):
        bi = core // 4
        h0 = 4 * (core % 4)
        qcols = [w_q[:, DH * (h0 + h):DH * (h0 + h + 1)][:, perm] for h in range(4)]
        kcols = [w_k[:, DH * (h0 + h):DH * (h0 + h + 1)][:, perm] for h in range(4)]
        wqk = np.ascontiguousarray(np.concatenate(qcols + kcols, axis=1))
        wv = np.ascontiguousarray(w_v[:, DH * h0:DH * (h0 + 4)])
        wo = np.ascontiguousarray(np.asarray(w_out)[DH * h0:DH * (h0 + 4), :]).astype(np.float32)
        xb = np.ascontiguousarray(np.asarray(x[bi])).astype(np.float32)
        in_maps.append({
            "x": xb,
            "xT": np.ascontiguousarray(xb.T),
            "wqk": wqk, "wv": wv, "wo": wo,
            "trig": trig,
            "cqkv": np.ascontiguousarray(
                np.concatenate([wqk.sum(axis=0), wv.sum(axis=0)])[None, :]),
        })
    return in_maps


def run(inputs, trace=False):
    if 'nc' not in _cache:
        _cache['nc'] = build()
    nc = _cache['nc']
    in_maps = _host_shards(**inputs)
    res = run_bass_kernel_spmd(nc, in_maps, core_ids=list(range(8)), trace=trace)
    parts = [res.results[i]["y"] for i in range(8)]
    y = np.stack([
        parts[0] + parts[1] + parts[2] + parts[3],
        parts[4] + parts[5] + parts[6] + parts[7],
    ]).astype(np.float32)
    return y, res


def kernel(**inputs):
    y, _ = run(inputs, trace=False)
    return y

